# revision 27
# baseline (speedup 1.0000x reference)
"""Self-contained Trainium2 kernel for the 6-layer transformer encoder.

Strategy: data-parallel over batch (16 items -> 2 per NeuronCore, 8 cores,
no collectives). Host does the embedding gather + positional encoding,
folds LN affine params into adjacent weight matrices, and re-tiles weights
into DMA-friendly blocks. The device kernel keeps the residual stream in
SBUF for all 6 layers; all matmuls run in bf16 on the PE with f32 PSUM
accumulation. Attention computes transposed scores S_T = K @ Q^T so the
softmax exp fuses into the PSUM->SBUF copyback and the denominator comes
free via a ones-column appended to V; softmax normalization is deferred
and applied batched per d-chunk. Activation tiles are half-token-width
([128, 512]) so phases pipeline at batch/token-half granularity.
"""

import math

import numpy as np
import ml_dtypes

import concourse.bass as bass
import concourse.mybir as mybir
import concourse.tile as tile
from concourse import bacc
from concourse.bass_utils import run_bass_kernel_spmd
from concourse.masks import make_identity

# Model dims (hardcoded per problem spec).
B, S, D, H, F, L, V = 16, 512, 1024, 16, 4096, 6, 32000
HD = D // H          # 64
EPS = 1e-5
NCORES = 8
BL = B // NCORES     # 2 local batch items per core
T = BL * S           # 1024 local tokens
P = 128
TC = T // P          # 8 token chunks
DC = D // P          # 8 feature chunks
FC = F // P          # 32 ffn chunks

BF = mybir.dt.bfloat16
F32 = mybir.dt.float32
AF = mybir.ActivationFunctionType
ALU = mybir.AluOpType

_CACHE = {}


# ----------------------------------------------------------------------------
# Device kernel
# ----------------------------------------------------------------------------

def _ps2(pools, n):
    """n accumulation groups out of ceil(n/2) two-bank PSUM tiles."""
    tiles = [pools["psum"].tile([P, 2, 512], F32, name="ps") for _ in range((n + 1) // 2)]
    return [tiles[g // 2][:, g % 2, :] for g in range(n)]


def _emit_layernorm_to_T(nc, pools, x_tiles, dst, ident):
    """LN (no affine) of x (8 x [P, D] f32, token-major) -> transposed bf16
    half tiles dst[dc][tg] ([P, S], feature-major)."""
    stats_p, hs_p, psum_p = pools["stats"], pools["hs"], pools["psum"]
    eps = pools["eps"]
    for tg in range(2):
        hs = []
        for tcc in range(4):
            t = tg * 4 + tcc
            st = stats_p.tile([P, 2, 6], F32, name="bnst")
            for sub in range(2):
                nc.vector.bn_stats(st[:, sub, :], x_tiles[t][:, sub * 512:(sub + 1) * 512])
            mv = stats_p.tile([P, 2], F32, name="bnmv")
            nc.vector.bn_aggr(mv, st)
            std = stats_p.tile([P, 1], F32, name="bnsd")
            nc.scalar.activation(std, mv[:, 1:2], AF.Sqrt, bias=eps)
            rstd = stats_p.tile([P, 1], F32, name="bnrs")
            nc.vector.reciprocal(rstd, std)
            nm = stats_p.tile([P, 1], F32, name="bnnm")
            nc.vector.tensor_scalar(nm, mv[:, 0:1], rstd, -1.0, ALU.mult, ALU.mult)
            h = hs_p.tile([P, D], BF, name="hs")
            # h = x * rstd - mean*rstd  (cast to bf16 on write), on ACT
            nc.scalar.activation(h, x_tiles[t], AF.Identity, bias=nm, scale=rstd)
            hs.append(h)
        for dc in range(DC):
            ps = psum_p.tile([P, 4, P], BF, name="ps")
            for j in range(4):
                nc.tensor.transpose(ps[:, j, :], hs[j][:, dc * P:(dc + 1) * P], ident)
            nc.scalar.activation(dst[dc][tg], ps, AF.Copy)


def _emit_proj_T(nc, pools, w_dram, b_sb, h_half, out_half, layer):
    """Transposed projection: out[m][th] = (W^T h^T) half tiles with bias per
    out-feature fused on the ACT copyback. w_dram layout [L, DC(k), DC(m), P, P]."""
    wq_p = pools["wq"]
    for th in range(2):
        psums = None
        for k in range(DC):
            wt = wq_p.tile([P, DC, P], BF, name="wqt")
            src = w_dram[layer, k].rearrange("m p c -> p m c")
            nc.sync.dma_start(out=wt, in_=src)
            if k == 0:
                psums = _ps2(pools, DC)
            for m in range(DC):
                nc.tensor.matmul(
                    psums[m],
                    wt[:, m, :],
                    h_half[k][th],
                    start=(k == 0),
                    stop=(k == DC - 1),
                )
        for m in range(DC):
            if m % 2 == 0:
                nc.scalar.activation(
                    out_half[m][th], psums[m], AF.Identity, bias=b_sb[:, m:m + 1]
                )
            else:
                nc.vector.tensor_scalar(
                    out_half[m][th], psums[m], b_sb[:, m:m + 1], None, ALU.add
                )


def _emit_proj_N(nc, pools, w_dram, lhs_half, layer, consume):
    """Normal-layout projection: out[t, o] = sum_k lhs_T[k] @ W[k].
    lhs_half[k][tg] are [P, S] tiles; consume(t, oh, psum) evicts."""
    wr_p = pools["wr"]
    for tg in range(2):
        psums = None
        for k in range(DC):
            wt = wr_p.tile([P, D], BF, name="wrt")
            nc.sync.dma_start(out=wt, in_=w_dram[layer, k])
            if k == 0:
                flat = _ps2(pools, 8)
                psums = [[flat[tcc * 2 + oh] for oh in range(2)] for tcc in range(4)]
            for tcc in range(4):
                for oh in range(2):
                    nc.tensor.matmul(
                        psums[tcc][oh],
                        lhs_half[k][tg][:, tcc * P:(tcc + 1) * P],
                        wt[:, oh * 512:(oh + 1) * 512],
                        start=(k == 0),
                        stop=(k == DC - 1),
                    )
        for tcc in range(4):
            for oh in range(2):
                consume(tg * 4 + tcc, oh, psums[tcc][oh])


def _half_tiles(pool, n, name):
    return [[pool.tile([P, S], BF, name=name) for _ in range(2)] for _ in range(n)]


def _emit_layer(nc, pools, x_tiles, wd, layer, ident, dumps=None):
    """One encoder layer, in-place on x_tiles."""
    hov_p, kh_p = pools["hov"], pools["kh"]
    at_p, va_p = pools["at"], pools["va"]
    psum_p, small_p = pools["psum"], pools["stats"]
    rb_p = pools["rb"]

    # --- LN1 -> h_T ---
    h_T = _half_tiles(hov_p, DC, "hovs")
    _emit_layernorm_to_T(nc, pools, x_tiles, h_T, ident)

    # per-layer fused biases (per out-feature partition layout)
    bq_sb = small_p.tile([P, DC], F32, name="bq")
    nc.sync.dma_start(out=bq_sb, in_=wd["bq"][layer])
    bk_sb = small_p.tile([P, DC], F32, name="bk")
    nc.sync.dma_start(out=bk_sb, in_=wd["bk"][layer])
    b1_sb = small_p.tile([P, FC], F32, name="b1")
    nc.sync.dma_start(out=b1_sb, in_=wd["b1"][layer])

    # --- QKV ---
    q_T = _half_tiles(pools["qr"], DC, "qrs")
    _emit_proj_T(nc, pools, wd["wq"], bq_sb, h_T, q_T, layer)
    k_T = _half_tiles(kh_p, DC, "khs")
    _emit_proj_T(nc, pools, wd["wk"], bk_sb, h_T, k_T, layer)

    v_N = [[hov_p.tile([P, S], BF, name="hovs") for _ in range(2)] for _ in range(TC)]

    def v_consume(t, oh, ps):
        if (t + oh) % 2 == 0:
            nc.scalar.activation(v_N[t][oh], ps, AF.Copy)
        else:
            nc.vector.tensor_copy(v_N[t][oh], ps)

    _emit_proj_N(nc, pools, wd["wv"], h_T, layer, v_consume)

    if dumps is not None:
        for i in range(DC):
            for g in range(2):
                nc.sync.dma_start(out=dumps["d_hT"][i, :, g * S:(g + 1) * S], in_=h_T[i][g])
                nc.sync.dma_start(out=dumps["d_qT"][i, :, g * S:(g + 1) * S], in_=q_T[i][g])
                nc.sync.dma_start(out=dumps["d_kT"][i, :, g * S:(g + 1) * S], in_=k_T[i][g])
            for g in range(2):
                nc.sync.dma_start(out=dumps["d_v"][i, :, g * S:(g + 1) * S], in_=v_N[i][g])

    # --- attention (unnormalized O; batched deferred softmax normalization) ---
    o_T = _half_tiles(hov_p, DC, "hovs")
    rscr = wd["rscr"][layer]
    av_tile = None
    for b in range(BL):
        denoms = pools["dn"].tile([H, S], F32, name="dn")
        for h in range(H):
            bh = b * H + h
            dc = h // 2
            po = (h % 2) * HD  # partition offset of this head's rows
            # V_aug: head slice of V with a ones column appended
            va = va_p.tile([P, 4, HD + 1], BF, name="va")
            for c in range(4):
                nc.vector.tensor_copy(
                    va[:, c, :HD],
                    v_N[4 * b + c][h // 8][:, (h % 8) * HD:(h % 8 + 1) * HD],
                )
            nc.vector.memset(va[:, :, HD:], 1.0)
            # scores (transposed) + fused exp (2 banks per psum tile)
            a_T = [at_p.tile([P, 2, S], BF, name="at") for _ in range(2)]
            for half in range(2):
                ps = psum_p.tile([P, 2, 512], F32, name="ps")
                for cc in range(2):
                    c = half * 2 + cc
                    nc.tensor.matmul(
                        ps[:, cc, :],
                        k_T[dc][b][po:po + HD, c * P:(c + 1) * P],
                        q_T[dc][b][po:po + HD, :],
                        start=True,
                        stop=True,
                    )
                nc.scalar.activation(
                    a_T[half], ps, AF.Exp, scale=1.0 / math.sqrt(HD)
                )
            # AV with denominator row; pairs of heads share a psum tile
            if bh % 2 == 0:
                av_tile = psum_p.tile([P, 2, 512], F32, name="ps")
            pso = av_tile[:, bh % 2, :]
            for c in range(4):
                nc.tensor.matmul(
                    pso[:HD + 1, :],
                    va[:, c, :],
                    a_T[c // 2][:, c % 2, :],
                    start=(c == 0),
                    stop=(c == 3),
                )
            if dumps is not None and bh < 4:
                for half in range(2):
                    nc.sync.dma_start(
                        out=dumps["d_aT"][bh, :, half * 2:half * 2 + 2, :],
                        in_=a_T[half])
            nc.vector.tensor_copy(o_T[dc][b][po:po + HD, :], pso[:HD, :])
            # denominator row -> partition-0 slot -> DMA into denoms[h]
            dslot = pools["rrow"].tile([1, S], F32, name="dsl")
            nc.vector.tensor_copy(dslot, pso[HD:HD + 1, :])
            nc.sync.dma_start(out=denoms[h:h + 1, :], in_=dslot)
        # per-batch reciprocal; b=0 normalization hides under b=1 attention
        rrec = pools["rr"].tile([H, S], F32, name="rr")
        nc.vector.reciprocal(rrec, denoms)
        rrbf = pools["rr"].tile([H, S], BF, name="rrb")
        nc.vector.tensor_copy(rrbf, rrec)
        nc.sync.dma_start(out=rscr[:, b * S:(b + 1) * S], in_=rrbf)
        for dc in range(DC):
            rbt = rb_p.tile([P, S], BF, name="rb")
            for half in range(2):
                h = 2 * dc + half
                nc.sync.dma_start(
                    out=rbt[half * HD:(half + 1) * HD, :],
                    in_=rscr[h:h + 1, b * S:(b + 1) * S].to_broadcast([HD, S]),
                )
            nc.vector.tensor_tensor(o_T[dc][b], o_T[dc][b], rbt, ALU.mult)

    if dumps is not None:
        for i in range(DC):
            for g in range(2):
                nc.sync.dma_start(out=dumps["d_oT"][i, :, g * S:(g + 1) * S], in_=o_T[i][g])

    # --- attn out proj + residual ---
    def proj_consume(t, oh, ps):
        xs = x_tiles[t][:, oh * 512:(oh + 1) * 512]
        nc.vector.tensor_tensor(xs, ps, xs, ALU.add)

    _emit_proj_N(nc, pools, wd["wo"], o_T, layer, proj_consume)

    if dumps is not None:
        for i in range(TC):
            nc.sync.dma_start(out=dumps["d_x1"][i], in_=x_tiles[i])

    # --- LN2 -> h2_T ---
    h2_T = _half_tiles(kh_p, DC, "khs")
    _emit_layernorm_to_T(nc, pools, x_tiles, h2_T, ident)

    # --- FFN1: r_T[f][th] = relu(W1^T h2_T + b1) ---
    r_T = _half_tiles(pools["qr"], FC, "qrs")
    w1_p = pools["wq"]
    for th in range(2):
        for fg in range(4):
            psums = None
            for k in range(DC):
                wt = w1_p.tile([P, DC, P], BF, name="wqt")
                src = wd["w1"][layer, k, fg * 8:(fg + 1) * 8].rearrange("f p c -> p f c")
                nc.sync.dma_start(out=wt, in_=src)
                if k == 0:
                    psums = _ps2(pools, DC)
                for f8 in range(DC):
                    nc.tensor.matmul(
                        psums[f8],
                        wt[:, f8, :],
                        h2_T[k][th],
                        start=(k == 0),
                        stop=(k == DC - 1),
                    )
            for f8 in range(DC):
                f = fg * 8 + f8
                if f8 % 2 == 0:
                    nc.scalar.activation(
                        r_T[f][th], psums[f8], AF.Relu, bias=b1_sb[:, f:f + 1]
                    )
                else:
                    nc.vector.tensor_scalar(
                        r_T[f][th], psums[f8], b1_sb[:, f:f + 1], 0.0,
                        ALU.add, ALU.max
                    )

    # --- FFN2 + residual ---
    wr_p = pools["wr"]
    for tg in range(2):
        psums = None
        for f in range(FC):
            wt = wr_p.tile([P, D], BF, name="wrt")
            nc.sync.dma_start(out=wt, in_=wd["w2"][layer, f])
            if f == 0:
                flat = _ps2(pools, 8)
                psums = [[flat[tcc * 2 + oh] for oh in range(2)] for tcc in range(4)]
            for tcc in range(4):
                for oh in range(2):
                    nc.tensor.matmul(
                        psums[tcc][oh],
                        r_T[f][tg][:, tcc * P:(tcc + 1) * P],
                        wt[:, oh * 512:(oh + 1) * 512],
                        start=(f == 0),
                        stop=(f == FC - 1),
                    )
        for tcc in range(4):
            t = tg * 4 + tcc
            for oh in range(2):
                xs = x_tiles[t][:, oh * 512:(oh + 1) * 512]
                nc.vector.tensor_tensor(xs, psums[tcc][oh], xs, ALU.add)


def build(n_layers=L, dump=False):
    key = ("nc", n_layers, dump)
    if key in _CACHE:
        return _CACHE[key]
    nc = bacc.Bacc("TRN2", target_bir_lowering=False, debug=False,
                   num_devices=NCORES)
    x0 = nc.dram_tensor("x0", [T, D], F32, kind="ExternalInput").ap()
    wd = {
        "wq": nc.dram_tensor("wq", [L, DC, DC, P, P], BF, kind="ExternalInput").ap(),
        "wk": nc.dram_tensor("wk", [L, DC, DC, P, P], BF, kind="ExternalInput").ap(),
        "wv": nc.dram_tensor("wv", [L, DC, P, D], BF, kind="ExternalInput").ap(),
        "wo": nc.dram_tensor("wo", [L, DC, P, D], BF, kind="ExternalInput").ap(),
        "w1": nc.dram_tensor("w1", [L, DC, FC, P, P], BF, kind="ExternalInput").ap(),
        "w2": nc.dram_tensor("w2", [L, FC, P, D], BF, kind="ExternalInput").ap(),
        "bq": nc.dram_tensor("bq", [L, P, DC], F32, kind="ExternalInput").ap(),
        "bk": nc.dram_tensor("bk", [L, P, DC], F32, kind="ExternalInput").ap(),
        "b1": nc.dram_tensor("b1", [L, P, FC], F32, kind="ExternalInput").ap(),
    }
    out = nc.dram_tensor("out", [T, D], F32, kind="ExternalOutput").ap()
    wd["rscr"] = nc.dram_tensor("rscr", [L, H, T], BF).ap()
    dumps = {}
    if dump:
        for nm, shp, dt in [
            ("d_hT", [DC, P, T], BF), ("d_qT", [DC, P, T], BF),
            ("d_kT", [DC, P, T], BF), ("d_v", [TC, P, D], BF),
            ("d_aT", [4, P, 4, S], BF), ("d_oT", [DC, P, T], BF),
            ("d_x1", [TC, P, D], F32),
        ]:
            dumps[nm] = nc.dram_tensor(nm, shp, dt, kind="ExternalOutput").ap()

    with tile.TileContext(nc) as tc:
        pools = {}
        import contextlib
        stack = contextlib.ExitStack()

        def pool(name, bufs, **kw):
            pools[name] = stack.enter_context(tc.tile_pool(name=name, bufs=bufs, **kw))

        pool("x", TC)            # 8 x [P,D] f32 = 32K
        pool("hov", 46)          # h_T/o_T/V 48 + slack, [P,S] bf16 = 50K
        pool("qr", 2 * FC)   # Q_T 16 / R_T 64 shared tag [P,S] bf16 = 65K
        pool("kh", 2 * DC + 1)   # K_T 16 / h2_T 16 [P,S] bf16 = 17K
        pool("hs", 4)            # [P,D] bf16 = 8K
        pool("at", 4)            # [P,2,S] bf16 = 6K
        pool("va", 2)            # [P,4,65] bf16 ~ 1K
        pool("wq", 3)            # [P,8,P] bf16 = 6K (shared wq/wk/w1)
        pool("wr", 3)            # [P,D] bf16 = 6K
        pool("stats", 8)        # small
        pool("dn", 2)            # [H,S] f32 = 4K
        pool("rr", 2)            # [H,S] f32+bf16 tags ~ 6K
        pool("rb", 2)            # [P,S] bf16 = 2K
        pool("rrow", 2)          # [1,S] f32 = 4K
        pool("single", 1)
        pool("psum", 4, space="PSUM")

        ident = pools["single"].tile([P, P], BF)
        make_identity(nc, ident)
        eps_t = pools["single"].tile([P, 1], F32)
        nc.vector.memset(eps_t, EPS)
        pools["eps"] = eps_t

        x_tiles = [pools["x"].tile([P, D], F32, name="x") for _ in range(TC)]
        for t in range(TC):
            nc.sync.dma_start(out=x_tiles[t], in_=x0[t * P:(t + 1) * P, :])

        for layer in range(n_layers):
            _emit_layer(nc, pools, x_tiles, wd, layer, ident,
                        dumps if (dump and layer == 0) else None)

        # final LN (affine applied on host)
        stats_p = pools["stats"]
        for t in range(TC):
            st = stats_p.tile([P, 2, 6], F32, name="bnst")
            for sub in range(2):
                nc.vector.bn_stats(st[:, sub, :], x_tiles[t][:, sub * 512:(sub + 1) * 512])
            mv = stats_p.tile([P, 2], F32, name="bnmv")
            nc.vector.bn_aggr(mv, st)
            std = stats_p.tile([P, 1], F32, name="bnsd")
            nc.scalar.activation(std, mv[:, 1:2], AF.Sqrt, bias=pools["eps"])
            rstd = stats_p.tile([P, 1], F32, name="bnrs")
            nc.vector.reciprocal(rstd, std)
            nm = stats_p.tile([P, 1], F32, name="bnnm")
            nc.vector.tensor_tensor(nm, mv[:, 0:1], rstd, ALU.mult)
            nc.vector.tensor_scalar(
                x_tiles[t], x_tiles[t], rstd, nm, ALU.mult, ALU.subtract)
            nc.sync.dma_start(out=out[t * P:(t + 1) * P, :], in_=x_tiles[t])
        stack.close()
    nc.compile()
    _CACHE[key] = nc
    return nc


# ----------------------------------------------------------------------------
# Host side
# ----------------------------------------------------------------------------

def _positional_encoding(seq_len, d_model):
    pos = np.arange(seq_len, dtype=np.float32)[:, None]
    div = np.exp(
        np.arange(0, d_model, 2, dtype=np.float32) * (-math.log(10000.0) / d_model)
    ).astype(np.float32)
    pe = np.zeros((seq_len, d_model), dtype=np.float32)
    pe[:, 0::2] = np.sin(pos * div)
    pe[:, 1::2] = np.cos(pos * div)
    return pe


def _host_prep(tokens, emb, params):
    tokens = np.asarray(tokens)
    emb = np.asarray(emb, dtype=np.float32)
    layers = params["layers"]
    g1 = np.asarray(layers["ln1_g"], np.float32)
    b1n = np.asarray(layers["ln1_b"], np.float32)
    g2 = np.asarray(layers["ln2_g"], np.float32)
    b2n = np.asarray(layers["ln2_b"], np.float32)
    Wq = np.asarray(layers["Wq"], np.float32)
    Wk = np.asarray(layers["Wk"], np.float32)
    Wv = np.asarray(layers["Wv"], np.float32)
    Wo = np.asarray(layers["Wo"], np.float32)
    W1 = np.asarray(layers["W1"], np.float32)
    W2 = np.asarray(layers["W2"], np.float32)
    bq = np.asarray(layers["bq"], np.float32)
    bk = np.asarray(layers["bk"], np.float32)
    bv = np.asarray(layers["bv"], np.float32)
    bo = np.asarray(layers["bo"], np.float32)
    b1 = np.asarray(layers["b1"], np.float32)
    b2 = np.asarray(layers["b2"], np.float32)

    # Fold LN affines into the adjacent projections.
    Wq_f = g1[:, :, None] * Wq
    Wk_f = g1[:, :, None] * Wk
    Wv_f = g1[:, :, None] * Wv
    W1_f = g2[:, :, None] * W1
    bq_f = bq + np.einsum("ld,ldo->lo", b1n, Wq)
    bk_f = bk + np.einsum("ld,ldo->lo", b1n, Wk)
    bv_f = bv + np.einsum("ld,ldo->lo", b1n, Wv)
    b1_f = b1 + np.einsum("ld,ldo->lo", b2n, W1)

    # This kernel build fuses only the per-partition-aligned biases
    # (bq, bk, b1). bv/bo/b2 are zero for this problem's init; verify.
    for name, arr in (("bv", bv_f), ("bo", bo), ("b2", b2)):
        assert np.abs(arr).max() < 1e-30, f"{name} nonzero; unsupported build"

    bf16 = ml_dtypes.bfloat16

    def to_lhsT_blocks(w, n_out_chunks):
        # [L, D, O] -> [L, DC(k), O/P(m), P, P]
        Lx, Din, Dout = w.shape
        return np.ascontiguousarray(
            w.reshape(Lx, DC, P, n_out_chunks, P).transpose(0, 1, 3, 2, 4)
        ).astype(bf16)

    wd_host = {
        "wq": to_lhsT_blocks(Wq_f, DC),
        "wk": to_lhsT_blocks(Wk_f, DC),
        "wv": np.ascontiguousarray(Wv_f.reshape(L, DC, P, D)).astype(bf16),
        "wo": np.ascontiguousarray(Wo.reshape(L, DC, P, D)).astype(bf16),
        "w1": to_lhsT_blocks(W1_f, FC),
        "w2": np.ascontiguousarray(W2.reshape(L, FC, P, D)).astype(bf16),
        "bq": np.ascontiguousarray(bq_f.reshape(L, DC, P).transpose(0, 2, 1)),
        "bk": np.ascontiguousarray(bk_f.reshape(L, DC, P).transpose(0, 2, 1)),
        "b1": np.ascontiguousarray(b1_f.reshape(L, FC, P).transpose(0, 2, 1)),
    }

    x0 = emb[tokens.astype(np.int64)] * math.sqrt(D) + _positional_encoding(S, D)
    x0 = x0.astype(np.float32)  # [B, S, D]
    return x0, wd_host


def kernel(tokens, emb, params):
    x0, wd_host = _host_prep(tokens, emb, params)
    nc = build()
    in_maps = []
    for c in range(NCORES):
        m = {"x0": np.ascontiguousarray(x0[c * BL:(c + 1) * BL].reshape(T, D))}
        m.update(wd_host)
        in_maps.append(m)
    res = run_bass_kernel_spmd(nc, in_maps, core_ids=list(range(NCORES)))
    outs = [res.results[c]["out"].reshape(BL, S, D) for c in range(NCORES)]
    y = np.concatenate(outs, axis=0)  # [B, S, D]
    lnf_g = np.asarray(params["lnf_g"], np.float32)
    lnf_b = np.asarray(params["lnf_b"], np.float32)
    return (y * lnf_g + lnf_b).astype(np.float32)


# revision 28
# speedup vs baseline: 1.0710x; 1.0710x over previous
"""Self-contained Trainium2 kernel for the 6-layer transformer encoder.

Strategy: data-parallel over batch (16 items -> 2 per NeuronCore, 8 cores,
no collectives). Host does the embedding gather + positional encoding,
folds LN affine params into adjacent weight matrices, and re-tiles weights
into DMA-friendly blocks. The device kernel keeps the residual stream in
SBUF for all 6 layers; all matmuls run in bf16 on the PE with f32 PSUM
accumulation. Attention computes transposed scores S_T = K @ Q^T so the
softmax exp fuses into the PSUM->SBUF copyback and the denominator comes
free via a ones-column appended to V; softmax normalization is deferred
and applied batched per d-chunk. Activation tiles are half-token-width
([128, 512]) so phases pipeline at batch/token-half granularity.
"""

import math

import numpy as np
import ml_dtypes

import concourse.bass as bass
import concourse.mybir as mybir
import concourse.tile as tile
from concourse import bacc
from concourse.bass_utils import run_bass_kernel_spmd
from concourse.masks import make_identity

# Model dims (hardcoded per problem spec).
B, S, D, H, F, L, V = 16, 512, 1024, 16, 4096, 6, 32000
HD = D // H          # 64
EPS = 1e-5
NCORES = 8
BL = B // NCORES     # 2 local batch items per core
T = BL * S           # 1024 local tokens
P = 128
TC = T // P          # 8 token chunks
DC = D // P          # 8 feature chunks
FC = F // P          # 32 ffn chunks

BF = mybir.dt.bfloat16
F32 = mybir.dt.float32
AF = mybir.ActivationFunctionType
ALU = mybir.AluOpType

_CACHE = {}


# ----------------------------------------------------------------------------
# Device kernel
# ----------------------------------------------------------------------------

def _ps2(pools, n):
    """n accumulation groups out of ceil(n/2) two-bank PSUM tiles."""
    tiles = [pools["psum"].tile([P, 2, 512], F32, name="ps") for _ in range((n + 1) // 2)]
    return [tiles[g // 2][:, g % 2, :] for g in range(n)]


def _emit_layernorm_to_T(nc, pools, x_tiles, dst, ident):
    """LN (no affine) of x (8 x [P, D] f32, token-major) -> transposed bf16
    half tiles dst[dc][tg] ([P, S], feature-major)."""
    stats_p, hs_p, psum_p = pools["stats"], pools["hs"], pools["psum"]
    eps = pools["eps"]
    for tg in range(2):
        hs = []
        for tcc in range(4):
            t = tg * 4 + tcc
            st = stats_p.tile([P, 2, 6], F32, name="bnst")
            for sub in range(2):
                nc.vector.bn_stats(st[:, sub, :], x_tiles[t][:, sub * 512:(sub + 1) * 512])
            mv = stats_p.tile([P, 2], F32, name="bnmv")
            nc.vector.bn_aggr(mv, st)
            std = stats_p.tile([P, 1], F32, name="bnsd")
            nc.scalar.activation(std, mv[:, 1:2], AF.Sqrt, bias=eps)
            rstd = stats_p.tile([P, 1], F32, name="bnrs")
            nc.vector.reciprocal(rstd, std)
            nm = stats_p.tile([P, 1], F32, name="bnnm")
            nc.vector.tensor_scalar(nm, mv[:, 0:1], rstd, -1.0, ALU.mult, ALU.mult)
            h = hs_p.tile([P, D], BF, name="hs")
            # h = x * rstd - mean*rstd  (cast to bf16 on write), on ACT
            nc.scalar.activation(h, x_tiles[t], AF.Identity, bias=nm, scale=rstd)
            hs.append(h)
        for dc in range(DC):
            ps = psum_p.tile([P, 4, P], BF, name="ps")
            for j in range(4):
                nc.tensor.transpose(ps[:, j, :], hs[j][:, dc * P:(dc + 1) * P], ident)
            nc.scalar.activation(dst[dc][tg], ps, AF.Copy)


def _emit_proj_T(nc, pools, w_dram, b_sb, h_half, out_half, layer, ths=(0, 1)):
    """Transposed projection: out[m][th] = (W^T h^T) half tiles with bias per
    out-feature fused on the ACT copyback. w_dram layout [L, DC(k), DC(m), P, P]."""
    wq_p = pools["wq"]
    for th in ths:
        psums = None
        for k in range(DC):
            wt = wq_p.tile([P, DC, P], BF, name="wqt")
            src = w_dram[layer, k].rearrange("m p c -> p m c")
            nc.sync.dma_start(out=wt, in_=src)
            if k == 0:
                psums = _ps2(pools, DC)
            for m in range(DC):
                nc.tensor.matmul(
                    psums[m],
                    wt[:, m, :],
                    h_half[k][th],
                    start=(k == 0),
                    stop=(k == DC - 1),
                )
        for m in range(DC):
            if m % 2 == 0:
                nc.scalar.activation(
                    out_half[m][th], psums[m], AF.Identity, bias=b_sb[:, m:m + 1]
                )
            else:
                nc.vector.tensor_scalar(
                    out_half[m][th], psums[m], b_sb[:, m:m + 1], None, ALU.add
                )


def _emit_proj_N(nc, pools, w_dram, lhs_half, layer, consume, tgs=(0, 1)):
    """Normal-layout projection: out[t, o] = sum_k lhs_T[k] @ W[k].
    lhs_half[k][tg] are [P, S] tiles; consume(t, oh, psum) evicts."""
    wr_p = pools["wr"]
    for tg in tgs:
        psums = None
        for k in range(DC):
            wt = wr_p.tile([P, D], BF, name="wrt")
            nc.sync.dma_start(out=wt, in_=w_dram[layer, k])
            if k == 0:
                flat = _ps2(pools, 8)
                psums = [[flat[tcc * 2 + oh] for oh in range(2)] for tcc in range(4)]
            for tcc in range(4):
                for oh in range(2):
                    nc.tensor.matmul(
                        psums[tcc][oh],
                        lhs_half[k][tg][:, tcc * P:(tcc + 1) * P],
                        wt[:, oh * 512:(oh + 1) * 512],
                        start=(k == 0),
                        stop=(k == DC - 1),
                    )
        for tcc in range(4):
            for oh in range(2):
                consume(tg * 4 + tcc, oh, psums[tcc][oh])


def _half_tiles(pool, n, name):
    return [[pool.tile([P, S], BF, name=name) for _ in range(2)] for _ in range(n)]


def _emit_layer(nc, pools, x_tiles, wd, layer, ident, dumps=None):
    """One encoder layer, in-place on x_tiles."""
    hov_p, kh_p = pools["hov"], pools["kh"]
    at_p, va_p = pools["at"], pools["va"]
    psum_p, small_p = pools["psum"], pools["stats"]
    rb_p = pools["rb"]

    # --- LN1 -> h_T ---
    h_T = _half_tiles(hov_p, DC, "hovs")
    _emit_layernorm_to_T(nc, pools, x_tiles, h_T, ident)

    # per-layer fused biases (per out-feature partition layout)
    bq_sb = small_p.tile([P, DC], F32, name="bq")
    nc.sync.dma_start(out=bq_sb, in_=wd["bq"][layer])
    bk_sb = small_p.tile([P, DC], F32, name="bk")
    nc.sync.dma_start(out=bk_sb, in_=wd["bk"][layer])
    b1_sb = small_p.tile([P, FC], F32, name="b1")
    nc.sync.dma_start(out=b1_sb, in_=wd["b1"][layer])

    # --- QKV (emitted per batch half, interleaved with attention below) ---
    q_T = _half_tiles(pools["qr"], DC, "qrs")
    k_T = _half_tiles(kh_p, DC, "khs")
    v_N = [[hov_p.tile([P, S], BF, name="hovs") for _ in range(2)] for _ in range(TC)]

    def v_consume(t, oh, ps):
        if (t + oh) % 2 == 0:
            nc.scalar.activation(v_N[t][oh], ps, AF.Copy)
        else:
            nc.vector.tensor_copy(v_N[t][oh], ps)

    def emit_qkv(bb):
        _emit_proj_T(nc, pools, wd["wq"], bq_sb, h_T, q_T, layer, ths=(bb,))
        _emit_proj_T(nc, pools, wd["wk"], bk_sb, h_T, k_T, layer, ths=(bb,))
        _emit_proj_N(nc, pools, wd["wv"], h_T, layer, v_consume, tgs=(bb,))

    emit_qkv(0)

    if dumps is not None:
        for i in range(DC):
            for g in range(2):
                nc.sync.dma_start(out=dumps["d_hT"][i, :, g * S:(g + 1) * S], in_=h_T[i][g])
                nc.sync.dma_start(out=dumps["d_qT"][i, :, g * S:(g + 1) * S], in_=q_T[i][g])
                nc.sync.dma_start(out=dumps["d_kT"][i, :, g * S:(g + 1) * S], in_=k_T[i][g])
            for g in range(2):
                nc.sync.dma_start(out=dumps["d_v"][i, :, g * S:(g + 1) * S], in_=v_N[i][g])

    # --- attention (unnormalized O; batched deferred softmax normalization) ---
    o_T = _half_tiles(hov_p, DC, "hovs")
    rscr = wd["rscr"][layer]
    av_tile = None
    for b in range(BL):
        if b == 1:
            emit_qkv(1)
        denoms = pools["dn"].tile([H, S], F32, name="dn")
        for h in range(H):
            bh = b * H + h
            dc = h // 2
            po = (h % 2) * HD  # partition offset of this head's rows
            # V_aug: head slice of V with a ones column appended
            va = va_p.tile([P, 4, HD + 1], BF, name="va")
            for c in range(4):
                nc.vector.tensor_copy(
                    va[:, c, :HD],
                    v_N[4 * b + c][h // 8][:, (h % 8) * HD:(h % 8 + 1) * HD],
                )
            nc.vector.memset(va[:, :, HD:], 1.0)
            # scores (transposed) + fused exp (2 banks per psum tile)
            a_T = [at_p.tile([P, 2, S], BF, name="at") for _ in range(2)]
            for half in range(2):
                ps = psum_p.tile([P, 2, 512], F32, name="ps")
                for cc in range(2):
                    c = half * 2 + cc
                    nc.tensor.matmul(
                        ps[:, cc, :],
                        k_T[dc][b][po:po + HD, c * P:(c + 1) * P],
                        q_T[dc][b][po:po + HD, :],
                        start=True,
                        stop=True,
                    )
                nc.scalar.activation(
                    a_T[half], ps, AF.Exp, scale=1.0 / math.sqrt(HD)
                )
            # AV with denominator row; pairs of heads share a psum tile
            if bh % 2 == 0:
                av_tile = psum_p.tile([P, 2, 512], F32, name="ps")
            pso = av_tile[:, bh % 2, :]
            for c in range(4):
                nc.tensor.matmul(
                    pso[:HD + 1, :],
                    va[:, c, :],
                    a_T[c // 2][:, c % 2, :],
                    start=(c == 0),
                    stop=(c == 3),
                )
            if dumps is not None and bh < 4:
                for half in range(2):
                    nc.sync.dma_start(
                        out=dumps["d_aT"][bh, :, half * 2:half * 2 + 2, :],
                        in_=a_T[half])
            nc.vector.tensor_copy(o_T[dc][b][po:po + HD, :], pso[:HD, :])
            # denominator row -> partition-0 slot -> DMA into denoms[h]
            dslot = pools["rrow"].tile([1, S], F32, name="dsl")
            nc.vector.tensor_copy(dslot, pso[HD:HD + 1, :])
            nc.sync.dma_start(out=denoms[h:h + 1, :], in_=dslot)
        # per-batch reciprocal; b=0 normalization hides under b=1 attention
        rrec = pools["rr"].tile([H, S], F32, name="rr")
        nc.vector.reciprocal(rrec, denoms)
        rrbf = pools["rr"].tile([H, S], BF, name="rrb")
        nc.vector.tensor_copy(rrbf, rrec)
        nc.sync.dma_start(out=rscr[:, b * S:(b + 1) * S], in_=rrbf)
        for dc in range(DC):
            rbt = rb_p.tile([P, S], BF, name="rb")
            for half in range(2):
                h = 2 * dc + half
                nc.sync.dma_start(
                    out=rbt[half * HD:(half + 1) * HD, :],
                    in_=rscr[h:h + 1, b * S:(b + 1) * S].to_broadcast([HD, S]),
                )
            nc.vector.tensor_tensor(o_T[dc][b], o_T[dc][b], rbt, ALU.mult)

    if dumps is not None:
        for i in range(DC):
            for g in range(2):
                nc.sync.dma_start(out=dumps["d_oT"][i, :, g * S:(g + 1) * S], in_=o_T[i][g])

    # --- attn out proj + residual ---
    def proj_consume(t, oh, ps):
        xs = x_tiles[t][:, oh * 512:(oh + 1) * 512]
        nc.vector.tensor_tensor(xs, ps, xs, ALU.add)

    _emit_proj_N(nc, pools, wd["wo"], o_T, layer, proj_consume)

    if dumps is not None:
        for i in range(TC):
            nc.sync.dma_start(out=dumps["d_x1"][i], in_=x_tiles[i])

    # --- LN2 -> h2_T ---
    h2_T = _half_tiles(kh_p, DC, "khs")
    _emit_layernorm_to_T(nc, pools, x_tiles, h2_T, ident)

    # --- FFN1: r_T[f][th] = relu(W1^T h2_T + b1) ---
    r_T = _half_tiles(pools["qr"], FC, "qrs")
    w1_p = pools["wq"]
    for th in range(2):
        for fg in range(4):
            psums = None
            for k in range(DC):
                wt = w1_p.tile([P, DC, P], BF, name="wqt")
                src = wd["w1"][layer, k, fg * 8:(fg + 1) * 8].rearrange("f p c -> p f c")
                nc.sync.dma_start(out=wt, in_=src)
                if k == 0:
                    psums = _ps2(pools, DC)
                for f8 in range(DC):
                    nc.tensor.matmul(
                        psums[f8],
                        wt[:, f8, :],
                        h2_T[k][th],
                        start=(k == 0),
                        stop=(k == DC - 1),
                    )
            for f8 in range(DC):
                f = fg * 8 + f8
                if f8 % 2 == 0:
                    nc.scalar.activation(
                        r_T[f][th], psums[f8], AF.Relu, bias=b1_sb[:, f:f + 1]
                    )
                else:
                    nc.vector.tensor_scalar(
                        r_T[f][th], psums[f8], b1_sb[:, f:f + 1], 0.0,
                        ALU.add, ALU.max
                    )

    # --- FFN2 + residual ---
    wr_p = pools["wr"]
    for tg in range(2):
        psums = None
        for f in range(FC):
            wt = wr_p.tile([P, D], BF, name="wrt")
            nc.sync.dma_start(out=wt, in_=wd["w2"][layer, f])
            if f == 0:
                flat = _ps2(pools, 8)
                psums = [[flat[tcc * 2 + oh] for oh in range(2)] for tcc in range(4)]
            for tcc in range(4):
                for oh in range(2):
                    nc.tensor.matmul(
                        psums[tcc][oh],
                        r_T[f][tg][:, tcc * P:(tcc + 1) * P],
                        wt[:, oh * 512:(oh + 1) * 512],
                        start=(f == 0),
                        stop=(f == FC - 1),
                    )
        for tcc in range(4):
            t = tg * 4 + tcc
            for oh in range(2):
                xs = x_tiles[t][:, oh * 512:(oh + 1) * 512]
                nc.vector.tensor_tensor(xs, psums[tcc][oh], xs, ALU.add)


def build(n_layers=L, dump=False):
    key = ("nc", n_layers, dump)
    if key in _CACHE:
        return _CACHE[key]
    nc = bacc.Bacc("TRN2", target_bir_lowering=False, debug=False,
                   num_devices=NCORES)
    x0 = nc.dram_tensor("x0", [T, D], F32, kind="ExternalInput").ap()
    wd = {
        "wq": nc.dram_tensor("wq", [L, DC, DC, P, P], BF, kind="ExternalInput").ap(),
        "wk": nc.dram_tensor("wk", [L, DC, DC, P, P], BF, kind="ExternalInput").ap(),
        "wv": nc.dram_tensor("wv", [L, DC, P, D], BF, kind="ExternalInput").ap(),
        "wo": nc.dram_tensor("wo", [L, DC, P, D], BF, kind="ExternalInput").ap(),
        "w1": nc.dram_tensor("w1", [L, DC, FC, P, P], BF, kind="ExternalInput").ap(),
        "w2": nc.dram_tensor("w2", [L, FC, P, D], BF, kind="ExternalInput").ap(),
        "bq": nc.dram_tensor("bq", [L, P, DC], F32, kind="ExternalInput").ap(),
        "bk": nc.dram_tensor("bk", [L, P, DC], F32, kind="ExternalInput").ap(),
        "b1": nc.dram_tensor("b1", [L, P, FC], F32, kind="ExternalInput").ap(),
    }
    out = nc.dram_tensor("out", [T, D], F32, kind="ExternalOutput").ap()
    wd["rscr"] = nc.dram_tensor("rscr", [L, H, T], BF).ap()
    dumps = {}
    if dump:
        for nm, shp, dt in [
            ("d_hT", [DC, P, T], BF), ("d_qT", [DC, P, T], BF),
            ("d_kT", [DC, P, T], BF), ("d_v", [TC, P, D], BF),
            ("d_aT", [4, P, 4, S], BF), ("d_oT", [DC, P, T], BF),
            ("d_x1", [TC, P, D], F32),
        ]:
            dumps[nm] = nc.dram_tensor(nm, shp, dt, kind="ExternalOutput").ap()

    with tile.TileContext(nc) as tc:
        pools = {}
        import contextlib
        stack = contextlib.ExitStack()

        def pool(name, bufs, **kw):
            pools[name] = stack.enter_context(tc.tile_pool(name=name, bufs=bufs, **kw))

        pool("x", TC)            # 8 x [P,D] f32 = 32K
        pool("hov", 46)          # h_T/o_T/V 48 + slack, [P,S] bf16 = 50K
        pool("qr", 2 * FC)   # Q_T 16 / R_T 64 shared tag [P,S] bf16 = 65K
        pool("kh", 2 * DC + 1)   # K_T 16 / h2_T 16 [P,S] bf16 = 17K
        pool("hs", 4)            # [P,D] bf16 = 8K
        pool("at", 4)            # [P,2,S] bf16 = 6K
        pool("va", 2)            # [P,4,65] bf16 ~ 1K
        pool("wq", 3)            # [P,8,P] bf16 = 6K (shared wq/wk/w1)
        pool("wr", 3)            # [P,D] bf16 = 6K
        pool("stats", 8)        # small
        pool("dn", 2)            # [H,S] f32 = 4K
        pool("rr", 2)            # [H,S] f32+bf16 tags ~ 6K
        pool("rb", 2)            # [P,S] bf16 = 2K
        pool("rrow", 2)          # [1,S] f32 = 4K
        pool("single", 1)
        pool("psum", 4, space="PSUM")

        ident = pools["single"].tile([P, P], BF)
        make_identity(nc, ident)
        eps_t = pools["single"].tile([P, 1], F32)
        nc.vector.memset(eps_t, EPS)
        pools["eps"] = eps_t

        x_tiles = [pools["x"].tile([P, D], F32, name="x") for _ in range(TC)]
        for t in range(TC):
            nc.sync.dma_start(out=x_tiles[t], in_=x0[t * P:(t + 1) * P, :])

        for layer in range(n_layers):
            _emit_layer(nc, pools, x_tiles, wd, layer, ident,
                        dumps if (dump and layer == 0) else None)

        # final LN (affine applied on host)
        stats_p = pools["stats"]
        for t in range(TC):
            st = stats_p.tile([P, 2, 6], F32, name="bnst")
            for sub in range(2):
                nc.vector.bn_stats(st[:, sub, :], x_tiles[t][:, sub * 512:(sub + 1) * 512])
            mv = stats_p.tile([P, 2], F32, name="bnmv")
            nc.vector.bn_aggr(mv, st)
            std = stats_p.tile([P, 1], F32, name="bnsd")
            nc.scalar.activation(std, mv[:, 1:2], AF.Sqrt, bias=pools["eps"])
            rstd = stats_p.tile([P, 1], F32, name="bnrs")
            nc.vector.reciprocal(rstd, std)
            nm = stats_p.tile([P, 1], F32, name="bnnm")
            nc.vector.tensor_tensor(nm, mv[:, 0:1], rstd, ALU.mult)
            nc.vector.tensor_scalar(
                x_tiles[t], x_tiles[t], rstd, nm, ALU.mult, ALU.subtract)
            nc.sync.dma_start(out=out[t * P:(t + 1) * P, :], in_=x_tiles[t])
        stack.close()
    nc.compile()
    _CACHE[key] = nc
    return nc


# ----------------------------------------------------------------------------
# Host side
# ----------------------------------------------------------------------------

def _positional_encoding(seq_len, d_model):
    pos = np.arange(seq_len, dtype=np.float32)[:, None]
    div = np.exp(
        np.arange(0, d_model, 2, dtype=np.float32) * (-math.log(10000.0) / d_model)
    ).astype(np.float32)
    pe = np.zeros((seq_len, d_model), dtype=np.float32)
    pe[:, 0::2] = np.sin(pos * div)
    pe[:, 1::2] = np.cos(pos * div)
    return pe


def _host_prep(tokens, emb, params):
    tokens = np.asarray(tokens)
    emb = np.asarray(emb, dtype=np.float32)
    layers = params["layers"]
    g1 = np.asarray(layers["ln1_g"], np.float32)
    b1n = np.asarray(layers["ln1_b"], np.float32)
    g2 = np.asarray(layers["ln2_g"], np.float32)
    b2n = np.asarray(layers["ln2_b"], np.float32)
    Wq = np.asarray(layers["Wq"], np.float32)
    Wk = np.asarray(layers["Wk"], np.float32)
    Wv = np.asarray(layers["Wv"], np.float32)
    Wo = np.asarray(layers["Wo"], np.float32)
    W1 = np.asarray(layers["W1"], np.float32)
    W2 = np.asarray(layers["W2"], np.float32)
    bq = np.asarray(layers["bq"], np.float32)
    bk = np.asarray(layers["bk"], np.float32)
    bv = np.asarray(layers["bv"], np.float32)
    bo = np.asarray(layers["bo"], np.float32)
    b1 = np.asarray(layers["b1"], np.float32)
    b2 = np.asarray(layers["b2"], np.float32)

    # Fold LN affines into the adjacent projections.
    Wq_f = g1[:, :, None] * Wq
    Wk_f = g1[:, :, None] * Wk
    Wv_f = g1[:, :, None] * Wv
    W1_f = g2[:, :, None] * W1
    bq_f = bq + np.einsum("ld,ldo->lo", b1n, Wq)
    bk_f = bk + np.einsum("ld,ldo->lo", b1n, Wk)
    bv_f = bv + np.einsum("ld,ldo->lo", b1n, Wv)
    b1_f = b1 + np.einsum("ld,ldo->lo", b2n, W1)

    # This kernel build fuses only the per-partition-aligned biases
    # (bq, bk, b1). bv/bo/b2 are zero for this problem's init; verify.
    for name, arr in (("bv", bv_f), ("bo", bo), ("b2", b2)):
        assert np.abs(arr).max() < 1e-30, f"{name} nonzero; unsupported build"

    bf16 = ml_dtypes.bfloat16

    def to_lhsT_blocks(w, n_out_chunks):
        # [L, D, O] -> [L, DC(k), O/P(m), P, P]
        Lx, Din, Dout = w.shape
        return np.ascontiguousarray(
            w.reshape(Lx, DC, P, n_out_chunks, P).transpose(0, 1, 3, 2, 4)
        ).astype(bf16)

    wd_host = {
        "wq": to_lhsT_blocks(Wq_f, DC),
        "wk": to_lhsT_blocks(Wk_f, DC),
        "wv": np.ascontiguousarray(Wv_f.reshape(L, DC, P, D)).astype(bf16),
        "wo": np.ascontiguousarray(Wo.reshape(L, DC, P, D)).astype(bf16),
        "w1": to_lhsT_blocks(W1_f, FC),
        "w2": np.ascontiguousarray(W2.reshape(L, FC, P, D)).astype(bf16),
        "bq": np.ascontiguousarray(bq_f.reshape(L, DC, P).transpose(0, 2, 1)),
        "bk": np.ascontiguousarray(bk_f.reshape(L, DC, P).transpose(0, 2, 1)),
        "b1": np.ascontiguousarray(b1_f.reshape(L, FC, P).transpose(0, 2, 1)),
    }

    x0 = emb[tokens.astype(np.int64)] * math.sqrt(D) + _positional_encoding(S, D)
    x0 = x0.astype(np.float32)  # [B, S, D]
    return x0, wd_host


def kernel(tokens, emb, params):
    x0, wd_host = _host_prep(tokens, emb, params)
    nc = build()
    in_maps = []
    for c in range(NCORES):
        m = {"x0": np.ascontiguousarray(x0[c * BL:(c + 1) * BL].reshape(T, D))}
        m.update(wd_host)
        in_maps.append(m)
    res = run_bass_kernel_spmd(nc, in_maps, core_ids=list(range(NCORES)))
    outs = [res.results[c]["out"].reshape(BL, S, D) for c in range(NCORES)]
    y = np.concatenate(outs, axis=0)  # [B, S, D]
    lnf_g = np.asarray(params["lnf_g"], np.float32)
    lnf_b = np.asarray(params["lnf_b"], np.float32)
    return (y * lnf_g + lnf_b).astype(np.float32)
